# revision 33
# baseline (speedup 1.0000x reference)
"""nn_CrossMamba Trainium2 kernel (v2).

Bidirectional Mamba over x = concat(context+seg_c, query+seg_q) (T=4096).
Sharding: 8 cores = (direction 2) x (batch 2) x (d_inner half 2); no
collectives — each core computes a partial out-projection over its 512
channels; the host sums partials and un-flips the backward direction.

v2 structure (per core, 4 time chunks of 1024):
  phase A: conv-folded in_proj matmuls -> silu -> xc (bf16); dbl PSUM
    accumulation; dt = -ln(sigmoid(-p)) batched; w = dt*xc; yacc init D*xc.
  phase B: per state s: B_s/C_s row broadcast via one-hot selector matmul
    (persistent SBUF selector) + PSUM->SBUF copies; per (s,j):
    dA=exp(A_s*lns) on ACT, dBx=w*bb on DVE (2x), scan on DVE or Pool
    (split for throughput), ym=h*cc on DVE, dual bf16 accumulators
    (DVE + Pool) to halve the serial chain.
  phase C: z matmuls + silu (deferred), merge/gate, out_proj, DMA out.
  Cross-chunk pipelining: chunk c+1's phase-A units are emitted interleaved
  with chunk c's phase-B states so every engine queue overlaps chunks.
"""

import sys

_TRN_REPO = "/opt/trn_rl_repo"
if _TRN_REPO not in sys.path:
    sys.path.insert(0, _TRN_REPO)

import numpy as np
import ml_dtypes

import concourse.bass as bass
import concourse.mybir as mybir
import concourse.tile as tile
from concourse import bacc
from concourse.bass import ds, ts

F32 = mybir.dt.float32
F32R = mybir.dt.float32r
BF16 = mybir.dt.bfloat16
AF = mybir.ActivationFunctionType
OP = mybir.AluOpType

T = 4096          # total time (Lc + Lq)
TC = 1024         # time chunk
NCH = T // TC
DM = 512          # d_model
DF = 1024         # d_inner full
DH = 512          # d_inner half (per core)
S = 16            # d_state
R = 32            # dt_rank
KC = 4            # d_conv
NKM = DM // 128   # 4  K-tiles for in_proj (contraction over d_model)
NDF = DF // 128   # 8  d-tiles full
NDH = DH // 128   # 4  d-tiles half
NMO = DM // 128   # 4  M-tiles for out_proj
NT = TC // 512    # 2  matmul N-chunks per time chunk

# ---- engine assignment tunables ----
# HW restrictions: TensorScalarPtr-class ops (scan, stt) are DVE-only;
# Pool cannot touch PSUM. Pool can still run SBUF TensorTensor/TensorCopy.
def _scan_on_pool(s, j):
    return False


# acc engine per (s, j): Pool takes most accumulate adds (plain TT)
def _acc_on_pool(s, j):
    return s % 8 != 1


# ym mul engine per (s, j): a few to Pool to balance DVE
def _ym_on_pool(s, j):
    return s % 8 == 7


# bb/cc copy engine per (s, n, which): PSUM-reading, so ACT/DVE only
_COPY_CYCLE = ("act", "dve", "act", "dve")


def build_program(stage="full", ablate=None):
    nc = bacc.Bacc("TRN2", target_bir_lowering=False, debug=False, num_devices=8)

    xT = nc.dram_tensor("xT", [DM, T + KC - 1], BF16, kind="ExternalInput")
    Win_l = nc.dram_tensor("Win_l", [DM, KC * DF + DH], BF16, kind="ExternalInput")
    convb = nc.dram_tensor("convb", [DF, 1], F32, kind="ExternalInput")
    Wx_l = nc.dram_tensor("Wx_l", [DF, R + 2 * S], BF16, kind="ExternalInput")
    Wdt_l = nc.dram_tensor("Wdt_l", [R + 1, DH], F32R, kind="ExternalInput")
    A_h = nc.dram_tensor("A_h", [DH, S], F32, kind="ExternalInput")
    D_h = nc.dram_tensor("D_h", [DH, 1], F32, kind="ExternalInput")
    Wout_l = nc.dram_tensor("Wout_l", [DH, DM], BF16, kind="ExternalInput")
    sel = nc.dram_tensor("sel", [2 * S, 2 * S * 128], BF16, kind="ExternalInput")
    ones_d = nc.dram_tensor("ones_d", [1, TC], F32R, kind="ExternalInput")

    if stage == "inproj":
        dbg = nc.dram_tensor("dbg", [DF + DH, T], F32R, kind="ExternalOutput")
    elif stage in ("conv", "dt"):
        dbg = nc.dram_tensor("dbg", [DF, T], F32R, kind="ExternalOutput")
    elif stage == "dbl":
        dbg = nc.dram_tensor("dbg", [R + 2 * S + 1, T], F32R, kind="ExternalOutput")
    elif stage == "scan":
        dbg = nc.dram_tensor("dbg", [DH, T], F32R, kind="ExternalOutput")
    else:
        outT = nc.dram_tensor("outT", [DM, T], F32R, kind="ExternalOutput")

    with tile.TileContext(nc) as tc:
        _emit(nc, tc, stage, locals(), ablate)
    nc.compile()
    return nc


def _emit(nc, tc, stage, tens, ablate=None):
    xT, Win_l, convb = tens["xT"], tens["Win_l"], tens["convb"]
    Wx_l, Wdt_l, A_h, D_h, Wout_l = (
        tens["Wx_l"], tens["Wdt_l"], tens["A_h"], tens["D_h"], tens["Wout_l"])
    sel_d = tens["sel"]
    dbg = tens.get("dbg")
    outT = tens.get("outT")
    dbgstage = stage not in ("full",)

    from contextlib import ExitStack
    ctx = ExitStack()
    with ctx:
        wpool = ctx.enter_context(tc.tile_pool(name="weights", bufs=1))
        xpool = ctx.enter_context(tc.tile_pool(name="xT", bufs=1))
        apool = ctx.enter_context(tc.tile_pool(name="phaseA", bufs=1))
        spool = ctx.enter_context(tc.tile_pool(name="scan", bufs=1))
        ypool = ctx.enter_context(tc.tile_pool(name="yout", bufs=1))
        hpool = ctx.enter_context(tc.tile_pool(name="hstate", bufs=1))
        ps_mm = ctx.enter_context(tc.tile_pool(name="psmm", bufs=2, space="PSUM"))
        ps_dbl = ctx.enter_context(tc.tile_pool(name="psdbl", bufs=2, space="PSUM"))
        ps_bc = ctx.enter_context(tc.tile_pool(name="psbc", bufs=4, space="PSUM"))

        # --- persistent weights in SBUF ---
        w_in = []
        for k in range(NKM):
            t_ = wpool.tile([128, KC * DF + DH], BF16, tag=f"win{k}", name=f"win{k}")
            nc.sync.dma_start(t_[:, :], Win_l[ts(k, 128), :])
            w_in.append(t_)
        w_x = []
        for k in range(NDF):
            t_ = wpool.tile([128, R + 2 * S], BF16, tag=f"wx{k}", name=f"wx{k}")
            nc.sync.dma_start(t_[:, :], Wx_l[ts(k, 128), :])
            w_x.append(t_)
        w_dt = wpool.tile([R + 1, DH], F32R, tag="wdt", name="wdt")
        nc.sync.dma_start(w_dt[:, :], Wdt_l[:, :])
        w_out = []
        for k in range(NDH):
            t_ = wpool.tile([128, DM], BF16, tag=f"wout{k}", name=f"wout{k}")
            nc.sync.dma_start(t_[:, :], Wout_l[ts(k, 128), :])
            w_out.append(t_)
        cb = []
        for k in range(NDF):
            t_ = wpool.tile([128, 1], F32, tag=f"cb{k}", name=f"cb{k}")
            nc.sync.dma_start(t_[:, :], convb[ts(k, 128), :])
            cb.append(t_)
        a_sb = []
        d_sb = []
        for k in range(NDH):
            t_ = wpool.tile([128, S], F32, tag=f"a{k}", name=f"a{k}")
            nc.sync.dma_start(t_[:, :], A_h[ts(k, 128), :])
            a_sb.append(t_)
            t_ = wpool.tile([128, 1], F32, tag=f"dd{k}", name=f"dd{k}")
            nc.sync.dma_start(t_[:, :], D_h[ts(k, 128), :])
            d_sb.append(t_)
        sel_sb = wpool.tile([2 * S, 2 * S * 128], BF16, tag="sel", name="sel_sb")
        nc.sync.dma_start(sel_sb[:, :], sel_d[:, :])
        # persistent scan state [128, S] per half d-tile
        hstate = [hpool.tile([128, S], F32, tag=f"hs{k}", name=f"hs{k}")
                  for k in range(NDH)]

        # ---------- helpers ----------

        def emit_xt_dma(c):
            xt = [xpool.tile([128, TC + KC - 1], BF16, tag=f"xt{k}", name=f"xt{k}",
                             bufs=2) for k in range(NKM)]
            for k in range(NKM):
                nc.sync.dma_start(xt[k][:, :], xT[ts(k, 128), ds(c * TC, TC + KC - 1)])
            return xt

        def phase_a_state(c, xt):
            """Allocate chunk-c phase-A tiles; return dict."""
            st = {}
            st["xt"] = xt
            st["xc"] = []      # 8 tiles, own half j<4 persistent tags
            st["dbl"] = [ps_dbl.tile([R + 2 * S, 512], F32, tag="dbl", name="dbl")
                         for _ in range(NT)]
            st["lns"] = []
            st["w"] = []
            st["yaccA"] = []
            st["yaccB"] = []
            return st

        def phase_a_unit(c, st, i, bias_gate=None):
            """Unit i in 0..15: j = i//2, n = i%2. in_proj + silu + dbl mm."""
            j, n = i // 2, i % 2
            xt = st["xt"]
            if n == 0:
                xc = apool.tile([128, TC], BF16, tag=f"xch{j}", name=f"xch{j}") \
                    if j < NDH else \
                    apool.tile([128, TC], BF16, tag="xcoth", name="xcoth", bufs=2)
                st["xc"].append(xc)
            xc = st["xc"][j]
            psn = ps_mm.tile([128, 512], F32, tag="mm", name="mm")
            for kk in range(KC):
                for k in range(NKM):
                    nc.tensor.matmul(
                        psn[:, :],
                        w_in[k][:, ds(kk * DF + j * 128, 128)],
                        xt[k][:, ds(kk + n * 512, 512)],
                        start=(kk == 0 and k == 0),
                        stop=(kk == KC - 1 and k == NKM - 1),
                    )
            # bias_gate (a copy of cb[j] made at a scan-block boundary) delays
            # the silu's readiness so the scheduler clusters silus together
            # instead of scattering them between exps (act-table thrash).
            bias = bias_gate if bias_gate is not None else cb[j][:, 0:1]
            nc.scalar.activation(xc[:, ds(n * 512, 512)], psn[:, :],
                                 AF.Silu, bias=bias)
            if stage in ("inproj", "conv"):
                nc.sync.dma_start(dbg[ts(j, 128), ds(c * TC + n * 512, 512)],
                                  xc[:, ds(n * 512, 512)])
            # dbl contribution (PSUM-accumulated over j)
            nc.tensor.matmul(
                st["dbl"][n][:, :], w_x[j][:, :],
                xc[:, ds(n * 512, 512)],
                start=(j == 0), stop=(j == NDF - 1),
            )

        def phase_a_post(c, st):
            """dbl collect, dt matmuls + batched sigmoid/ln, w stt, yacc init."""
            scanin = apool.tile([R + 1, TC], F32R, tag="scanin", name="scanin",
                                bufs=2)
            bc_sb = apool.tile([2 * S, TC], BF16, tag="bcsb", name="bcsb", bufs=2)
            if c < 2:
                nc.sync.dma_start(scanin[R:R + 1, :], tens["ones_d"][:, :])
            for n in range(NT):
                nc.vector.tensor_copy(scanin[0:R, ds(n * 512, 512)],
                                      st["dbl"][n][0:R, :])
                nc.vector.tensor_copy(bc_sb[:, ds(n * 512, 512)],
                                      st["dbl"][n][R:R + 2 * S, :])
            st["bc_sb"] = bc_sb
            if stage == "dbl":
                nc.sync.dma_start(dbg[0:R + 1, ds(c * TC, TC)], scanin[:, :])
                nc.sync.dma_start(dbg[R + 1:R + 1 + 2 * S, ds(c * TC, TC)],
                                  bc_sb[:, :])
            # dt: p = Wdt @ [dblR; 1]; sigm = sigmoid(-p) (f32r scratch);
            # lns = ln(sigm) (bf16)
            sgs = []
            for j in range(NDH):
                for n in range(NT):
                    psn = ps_mm.tile([128, 512], F32, tag="mm", name="mm")
                    nc.tensor.matmul(
                        psn[:, :], w_dt[:, ds(j * 128, 128)],
                        scanin[0:R + 1, ds(n * 512, 512)],
                        start=True, stop=True,
                    )
                    sg = apool.tile([128, 512], F32R, tag="sg", name="sg", bufs=6)
                    nc.scalar.activation(sg[:, :], psn[:, :], AF.Sigmoid,
                                         scale=-1.0)
                    sgs.append(sg)
            # zero-gate on the last sigmoid clusters the lns after all
            # sigmoids (one table switch instead of per-j ping-pong)
            # gate on sigmoid #5 (not the last): slots 6/7 are only freed by
            # gated lns, so gating on the last sigmoid would deadlock
            lgate = apool.tile([128, 1], F32, tag="lgate", name="lgate", bufs=2)
            nc.scalar.activation(lgate[:, :], sgs[5][:, 0:1], AF.Identity,
                                 scale=0.0)
            for j in range(NDH):
                lns = apool.tile([128, TC], BF16, tag=f"lns{j}", name=f"lns{j}",
                                 bufs=2)
                for n in range(NT):
                    nc.scalar.activation(lns[:, ds(n * 512, 512)],
                                         sgs[j * NT + n][:, :], AF.Ln,
                                         bias=lgate[:, 0:1])
                st["lns"].append(lns)
                if stage == "dt":
                    nc.sync.dma_start(dbg[ts(j, 128), ds(c * TC, TC)], lns[:, :])
            for j in range(NDH):
                # w = dt * xc = (-lns) * xc
                w = apool.tile([128, TC], BF16, tag=f"w{j}", name=f"w{j}", bufs=2)
                nc.vector.scalar_tensor_tensor(w[:, :], st["lns"][j][:, :], -1.0,
                                               st["xc"][j][:, :],
                                               op0=OP.mult, op1=OP.mult)
                st["w"].append(w)
                # dual accumulators; A initialized with the D*xc skip term
                ya = ypool.tile([128, TC], BF16, tag=f"ya{j}", name=f"ya{j}")
                nc.scalar.activation(ya[:, :], st["xc"][j][:, :], AF.Identity,
                                     scale=d_sb[j][:, 0:1])
                st["yaccA"].append(ya)
                yb = ypool.tile([128, TC], BF16, tag=f"yb{j}", name=f"yb{j}")
                st["yaccB"].append(yb)

        def phase_b_stateiter(c, st, s):
            """Broadcast B_s/C_s; per j: dA, dBx, scan, hstate, ym, acc."""
            bb = spool.tile([128, TC], BF16, tag=f"bb{s % 2}", name="bb", bufs=2)
            cc = spool.tile([128, TC], BF16, tag=f"cc{s % 2}", name="cc", bufs=2)
            cp = 0
            for which, dst in ((0, bb), (1, cc)):
                for n in range(NT):
                    pb = ps_bc.tile([128, 512], F32, tag="bc", name="bc")
                    nc.tensor.matmul(
                        pb[:, :],
                        sel_sb[:, ts(which * S + s, 128)],
                        st["bc_sb"][:, ds(n * 512, 512)],
                        start=True, stop=True)
                    eng = _COPY_CYCLE[(s * 4 + cp) % len(_COPY_CYCLE)]
                    if eng == "act":
                        nc.scalar.copy(dst[:, ds(n * 512, 512)], pb[:, :])
                    elif eng == "dve":
                        nc.vector.tensor_copy(dst[:, ds(n * 512, 512)], pb[:, :])
                    else:
                        nc.gpsimd.tensor_copy(dst[:, ds(n * 512, 512)], pb[:, :])
                    cp += 1
            st["cur_bb"] = bb
            for j in range(NDH):
                dA = spool.tile([128, TC], BF16, tag="dA", name="dA", bufs=6)
                nc.scalar.activation(dA[:, :], st["lns"][j][:, :], AF.Exp,
                                     scale=a_sb[j][:, s:s + 1])
                st["last_dA"] = dA
                dBx = spool.tile([128, TC], BF16, tag="dBx", name="dBx", bufs=3)
                nc.vector.tensor_tensor(dBx[:, :], st["w"][j][:, :], bb[:, :],
                                        op=OP.mult)
                h = spool.tile([128, TC], BF16, tag="h", name="h", bufs=3)
                init = 0.0 if c == 0 else hstate[j][:, s:s + 1]
                seng = nc.gpsimd if _scan_on_pool(s, j) else nc.vector
                if ablate == "noscan":
                    seng.tensor_tensor(h[:, :], dA[:, :], dBx[:, :], op=OP.mult)
                else:
                    seng.tensor_tensor_scan(h[:, :], dA[:, :], dBx[:, :],
                                            init, op0=OP.mult, op1=OP.add)
                nc.gpsimd.tensor_copy(hstate[j][:, s:s + 1], h[:, TC - 1:TC])
                ym = spool.tile([128, TC], BF16, tag="ym", name="ym", bufs=3)
                meng = nc.gpsimd if _ym_on_pool(s, j) else nc.vector
                meng.tensor_tensor(ym[:, :], h[:, :], cc[:, :], op=OP.mult)
                # dual accumulation: even s -> A (DVE), odd s -> B (Pool slot
                # but engine still chosen by _acc_on_pool)
                acc = st["yaccA"][j] if s % 2 == 0 else st["yaccB"][j]
                aeng = nc.gpsimd if _acc_on_pool(s, j) else nc.vector
                if s == 1:
                    # first write of yaccB: plain copy of ym
                    nc.vector.tensor_copy(st["yaccB"][j][:, :], ym[:, :])
                else:
                    aeng.tensor_tensor(acc[:, :], acc[:, :], ym[:, :], op=OP.add)

        def phase_c(c, st):
            """z (deferred) + merge/gate + out_proj + DMA."""
            xt = st["xt"]
            # zero-gate on the chunk's last dA clusters the z-silus after the
            # exp stream has drained (avoids mid-stream table reloads)
            zgate = ypool.tile([128, 1], F32, tag="zgate", name="zgate", bufs=2)
            nc.scalar.activation(zgate[:, :], st["last_dA"][:, 0:1], AF.Identity,
                                 scale=0.0)
            yg = []
            for j in range(NDH):
                z = apool.tile([128, TC], BF16, tag="zg", name="zg", bufs=2)
                for n in range(NT):
                    psn = ps_mm.tile([128, 512], F32, tag="mm", name="mm")
                    for k in range(NKM):
                        nc.tensor.matmul(
                            psn[:, :],
                            w_in[k][:, ds(KC * DF + j * 128, 128)],
                            xt[k][:, ds(KC - 1 + n * 512, 512)],
                            start=(k == 0), stop=(k == NKM - 1),
                        )
                    nc.scalar.activation(z[:, ds(n * 512, 512)], psn[:, :],
                                         AF.Silu, bias=zgate[:, 0:1])
                # y = (yaccA + yaccB) in place; then gate y *= silu(z)
                ya = st["yaccA"][j]
                nc.vector.tensor_tensor(ya[:, :], ya[:, :],
                                        st["yaccB"][j][:, :], op=OP.add)
                if stage == "scan":
                    ygf = ypool.tile([128, TC], F32R, tag="ygf", name="ygf",
                                     bufs=2)
                    nc.vector.tensor_copy(ygf[:, :], ya[:, :])
                    nc.sync.dma_start(dbg[ts(j, 128), ds(c * TC, TC)], ygf[:, :])
                    yg.append(ya)
                    continue
                nc.vector.tensor_tensor(ya[:, :], ya[:, :], z[:, :], op=OP.mult)
                yg.append(ya)
            if dbgstage:
                return
            for m in range(NMO):
                for n in range(NT):
                    pso = ps_mm.tile([128, 512], F32, tag="mm", name="mm")
                    for k in range(NDH):
                        nc.tensor.matmul(
                            pso[:, :], w_out[k][:, ds(m * 128, 128)],
                            yg[k][:, ds(n * 512, 512)],
                            start=(k == 0), stop=(k == NDH - 1),
                        )
                    osb = ypool.tile([128, 512], F32R, tag="osb", name="osb",
                                     bufs=2)
                    nc.scalar.copy(osb[:, :], pso[:, :])
                    nc.sync.dma_start(
                        outT[ts(m, 128), ds(c * TC + n * 512, 512)], osb[:, :])

        # ---------- main pipelined loop ----------
        xt0 = emit_xt_dma(0)
        st = phase_a_state(0, xt0)
        for i in range(2 * NDF):
            phase_a_unit(0, st, i)
        phase_a_post(0, st)

        for c in range(NCH):
            nxt = None
            unit_iter = None
            if c + 1 < NCH:
                xtn = emit_xt_dma(c + 1)
                nxt = phase_a_state(c + 1, xtn)
                unit_iter = iter(range(2 * NDF))
            if dbgstage and stage in ("inproj", "conv", "dbl", "dt"):
                # staged debug: run phase A only
                if nxt is not None:
                    for i in range(2 * NDF):
                        phase_a_unit(c + 1, nxt, i)
                    phase_a_post(c + 1, nxt)
                    st = nxt
                continue
            for s in range(S):
                phase_b_stateiter(c, st, s)
                # interleave next chunk's phase-A in blocks of 4 units; gate
                # the silus on this scan block so they cluster in the ACT queue
                if unit_iter is not None and s % 4 == 3:
                    g = s // 4
                    gates = {}
                    for jj in (2 * g, 2 * g + 1):
                        gt = spool.tile([128, 1], F32, tag=f"gcb{jj % 2}",
                                        name="gcb", bufs=2)
                        nc.scalar.activation(gt[:, :], st["cur_bb"][:, 0:1],
                                             AF.Identity, bias=cb[jj][:, 0:1],
                                             scale=0.0)
                        gates[jj] = gt
                    for _ in range(4):
                        i = next(unit_iter)
                        phase_a_unit(c + 1, nxt, i,
                                     bias_gate=gates[i // 2][:, 0:1])
            phase_c(c, st)
            if nxt is not None:
                phase_a_post(c + 1, nxt)
                st = nxt


# ---------------------------------------------------------------------------
# host side
# ---------------------------------------------------------------------------

_COMPILED = {}

# one-hot selector: sel[r, s*128 + m] = (r == s), for the B/C row broadcast
_SEL = np.zeros((2 * S, 2 * S * 128), np.float32)
for _s in range(2 * S):
    _SEL[_s, _s * 128:(_s + 1) * 128] = 1.0


class _CompiledSpmd:
    def __init__(self, nc, n_cores=8):
        import jax
        from jax.sharding import Mesh, PartitionSpec
        from jax.experimental.shard_map import shard_map
        from concourse.bass2jax import (
            _bass_exec_p, partition_id_tensor, install_neuronx_cc_hook)

        install_neuronx_cc_hook()
        self.jax = jax
        self.nc = nc
        self.n_cores = n_cores
        in_names, out_names, out_avals, zero_outs = [], [], [], []
        partition_name = nc.partition_id_tensor.name if nc.partition_id_tensor else None
        for alloc in nc.m.functions[0].allocations:
            if not isinstance(alloc, mybir.MemoryLocationSet):
                continue
            name = alloc.memorylocations[0].name
            if alloc.kind == "ExternalInput":
                if name != partition_name:
                    in_names.append(name)
            elif alloc.kind == "ExternalOutput":
                shape = tuple(alloc.tensor_shape)
                dtype = mybir.dt.np(alloc.dtype)
                out_avals.append(jax.core.ShapedArray(shape, dtype))
                out_names.append(name)
                zero_outs.append(np.zeros(shape, dtype))
        assert nc.dbg_addr is None
        self.in_names, self.out_names = in_names, out_names
        self.out_avals, self.zero_outs = out_avals, zero_outs
        all_in = list(in_names) + list(out_names)
        if partition_name is not None:
            all_in.append(partition_name)

        def _body(*args):
            operands = list(args)
            if partition_name is not None:
                operands.append(partition_id_tensor())
            return tuple(_bass_exec_p.bind(
                *operands,
                out_avals=tuple(out_avals), in_names=tuple(all_in),
                out_names=tuple(out_names),
                lowering_input_output_aliases=(),
                sim_require_finite=True, sim_require_nnan=True, nc=nc))

        devices = jax.devices()[:n_cores]
        mesh = Mesh(np.asarray(devices), ("core",))
        n_outs = len(out_avals)
        self.fn = jax.jit(
            shard_map(_body, mesh=mesh,
                      in_specs=(PartitionSpec("core"),) * (len(in_names) + n_outs),
                      out_specs=(PartitionSpec("core"),) * n_outs,
                      check_rep=False),
            keep_unused=True)
        self._zero_dev = None

    def run(self, in_maps):
        jax = self.jax
        concat = [np.concatenate([np.asarray(in_maps[c][nm])
                                  for c in range(self.n_cores)], axis=0)
                  for nm in self.in_names]
        if self._zero_dev is None:
            self._zero_dev = [
                jax.device_put(np.zeros((self.n_cores * z.shape[0], *z.shape[1:]),
                                        z.dtype))
                for z in self.zero_outs]
        args = [jax.device_put(a) for a in concat] + self._zero_dev
        outs = self.fn(*args)
        jax.block_until_ready(outs)
        return outs

    def results(self, outs):
        res = []
        for c in range(self.n_cores):
            d = {}
            for i, nm in enumerate(self.out_names):
                d[nm] = np.asarray(outs[i]).reshape(
                    self.n_cores, *self.out_avals[i].shape)[c]
            res.append(d)
        return res


def _get_compiled(stage="full", ablate=None):
    key = (stage, ablate)
    if key not in _COMPILED:
        nc = build_program(stage, ablate)
        _COMPILED[key] = _CompiledSpmd(nc, 8)
    return _COMPILED[key]


def make_in_maps(**inputs):
    """Build the 8 per-core input dicts from full inputs."""
    inp = {k: np.asarray(v, np.float32) for k, v in inputs.items()}
    x = np.concatenate([inp["context"] + inp["seg_context"],
                        inp["query"] + inp["seg_query"]], axis=1)  # [2, T, 512]
    W_in, conv_w, conv_b = inp["W_in"], inp["conv_w"], inp["conv_b"]
    W_x, W_dt, b_dt = inp["W_x"], inp["W_dt"], inp["b_dt"]
    negA = np.exp(inp["A_log"])  # = -A; dA = exp(A*dt) = exp(negA * ln s)
    D, W_out = inp["D"], inp["W_out"]
    Win_x, Win_z = W_in[:DF], W_in[DF:]

    in_maps = []
    metas = []
    for core in range(8):
        dirn, b, half = core // 4, (core // 2) % 2, core % 2
        xb = x[b] if dirn == 0 else x[b, ::-1]
        sl = slice(half * DH, (half + 1) * DH)
        # reorder d_inner so this core's half occupies channel blocks 0..3
        idx_half = np.arange(half * DH, (half + 1) * DH)
        idx_oth = np.arange((1 - half) * DH, (2 - half) * DH)
        perm = np.concatenate([idx_half, idx_oth])
        conv_blocks = [np.ascontiguousarray((Win_x * conv_w[:, k:k + 1]).T[:, perm])
                       for k in range(KC)]
        xpad = np.concatenate([np.zeros((DM, KC - 1), np.float32), xb.T], 1)
        m = {
            "xT": xpad.astype(ml_dtypes.bfloat16),
            "Win_l": np.concatenate(
                conv_blocks + [Win_z.T[:, sl]], 1).astype(ml_dtypes.bfloat16),
            "convb": np.ascontiguousarray(conv_b[perm, None]),
            "Wx_l": np.ascontiguousarray(W_x.T[perm]).astype(ml_dtypes.bfloat16),
            "Wdt_l": np.ascontiguousarray(
                np.concatenate([W_dt[sl].T, b_dt[None, sl]], 0)),
            "A_h": np.ascontiguousarray(negA[sl]),
            "D_h": np.ascontiguousarray(D[sl, None]),
            "Wout_l": np.ascontiguousarray(W_out[:, sl].T).astype(ml_dtypes.bfloat16),
            "sel": _SEL.astype(ml_dtypes.bfloat16),
            "ones_d": np.ones((1, TC), np.float32),
        }
        in_maps.append(m)
        metas.append((dirn, b, half))
    return in_maps, metas


def kernel(**inputs):
    Lc = np.asarray(inputs["context"]).shape[1]
    in_maps, metas = make_in_maps(**inputs)
    k = _get_compiled("full")
    outs = k.run(in_maps)
    res = k.results(outs)
    out = np.zeros((2, T - Lc, DM), np.float32)
    acc = {}
    for core, (dirn, b, half) in enumerate(metas):
        acc.setdefault((dirn, b), np.zeros((DM, T), np.float32))
        acc[(dirn, b)] += res[core]["outT"]
    for b in range(2):
        yf = acc[(0, b)].T
        yb = acc[(1, b)].T[::-1]
        out[b] = (0.5 * (yf + yb))[Lc:]
    return out.astype(np.float32)
